# revision 4
# baseline (speedup 1.0000x reference)
"""Trainium2 Bass kernel for nn_ConvModule: LN -> 1x1 conv (D->2I) -> SwiGLU
-> depthwise conv (K=31) -> PReLU -> 1x1 conv (I->D).

Sharding: data-parallel over batch, 2 batches per core across 8 cores.

v2 changes vs baseline (800us):
  - Weight preprocessing (transpose, ln_g fold, b1' = b1 + W1@ln_b) moved to
    host numpy; device setup is just DMA + f32->bf16 casts.
  - Whole GEMM path in bf16 (fp32 moving operands stream at half rate on the
    PE; bf16 also enables FWL weight loads and 1-pass transposes).
  - SwiGLU epilogue via ACT Silu (one op) + ACT Identity-with-bias + one DVE
    bf16 tensor_mul, instead of sigmoid + two fp32 scalar_tensor_tensor.
  - Depthwise conv: diag-matrix build on GPSIMD (idle engine), PE taps as
    before, DVE taps moved to 4B-aligned strip offsets so the bf16 chain runs
    in 2x DVE mode, PReLU+dwb-bias done in one ACT Prelu op.
  - LN mean/E[x2] folded into the ACT accumulation scales.
"""

import sys

sys.path.insert(0, "/opt/trn_rl_repo")

from contextlib import ExitStack

import numpy as np

import concourse.bacc as bacc
import concourse.tile as tile
from concourse import mybir
from concourse.masks import make_identity
from concourse.bass_utils import run_bass_kernel_spmd

B, T, D, I, K = 16, 2048, 512, 1024, 31
NCORES = 8
BPC = B // NCORES  # batches per core
E = 2 * I  # 2048
NTOK = BPC * T  # tokens per core
TP = T // 512  # time panels per batch (4)
ETILES = E // 128  # 16
CB = I // 128  # channel blocks (8)
DCH = D // 128  # d chunks (4)
PADL = 16  # left pad: tap k reads strip offset k+1, odd k -> even offset
PADR = 16
STRIPW = PADL + T + PADR  # 2080
# Tap split: DVE takes odd taps (even strip offsets -> 4B-aligned bf16 2x),
# PE takes the rest as diagonal matmuls.
NDVE = 13
DVE_TAPS = [2 * i + 1 for i in range(NDVE)]  # 1,3,...,25
PE_TAPS = [k for k in range(K) if k not in DVE_TAPS]  # 18 taps

F32 = mybir.dt.float32
BF16 = mybir.dt.bfloat16
ALU = mybir.AluOpType
ACTF = mybir.ActivationFunctionType
P = 128


def _build_kernel(ctx, tc):
    nc = tc.nc
    x_d = nc.dram_tensor("x", [BPC, T, D], F32, kind="ExternalInput").ap()
    # host-preprocessed weights
    w1t_d = nc.dram_tensor("w1t", [D, E], F32, kind="ExternalInput").ap()
    w2t_d = nc.dram_tensor("w2t", [I, D], F32, kind="ExternalInput").ap()
    b1p_d = nc.dram_tensor("b1p", [P, ETILES], F32, kind="ExternalInput").ap()
    dwsb_d = nc.dram_tensor("dwsb", [P, CB * K], F32, kind="ExternalInput").ap()
    dwb_d = nc.dram_tensor("dwbp", [P, CB], F32, kind="ExternalInput").ap()
    alpha_d = nc.dram_tensor("alphap", [P, CB], F32, kind="ExternalInput").ap()
    b2_d = nc.dram_tensor("b2row", [1, D], F32, kind="ExternalInput").ap()
    out_d = nc.dram_tensor("out", [BPC, T, D], F32, kind="ExternalOutput").ap()

    const = ctx.enter_context(tc.tile_pool(name="const", bufs=1))
    psum = ctx.enter_context(tc.tile_pool(name="psum", bufs=8, space="PSUM"))

    ident_bf = const.tile([P, P], BF16, tag="ident_bf")
    make_identity(nc, ident_bf[:])

    # ---- parameter tiles ----
    b1p = const.tile([P, ETILES], F32, tag="b1p")
    nc.sync.dma_start(b1p[:], b1p_d)
    alpha_sb = const.tile([P, CB], F32, tag="alpha_sb")
    nc.sync.dma_start(alpha_sb[:], alpha_d)
    dwb_sb = const.tile([P, CB], F32, tag="dwb_sb")
    nc.sync.dma_start(dwb_sb[:], dwb_d)
    dw_sb = const.tile([P, CB * K], F32, tag="dw_sb")
    nc.sync.dma_start(dw_sb[:], dwsb_d)
    eps_t = const.tile([P, 1], F32, tag="eps_t")
    nc.vector.memset(eps_t[:], 1e-5)
    ones_bf = const.tile([1, P], BF16, tag="ones_bf")
    nc.vector.memset(ones_bf[:], 1.0)

    w1t = [const.tile([P, E], BF16, tag=f"w1t{j}", name=f"w1t{j}") for j in range(DCH)]
    w2t = [const.tile([P, D], BF16, tag=f"w2t{i}", name=f"w2t{i}") for i in range(CB)]
    b2row_bf = const.tile([1, D], BF16, tag="b2row_bf")
    with tc.tile_pool(name="setup", bufs=2) as setup:
        b2f = setup.tile([1, D], F32, tag="b2f", bufs=1)
        nc.sync.dma_start(b2f[:], b2_d)
        nc.vector.tensor_copy(b2row_bf[:], b2f[:])
        for j in range(DCH):
            wst = setup.tile([P, E], F32, tag="wst", bufs=2)
            (nc.sync if j % 2 == 0 else nc.scalar).dma_start(
                wst[:], w1t_d[j * P:(j + 1) * P, :])
            nc.vector.tensor_copy(w1t[j][:], wst[:])
        for i in range(CB):
            wst2 = setup.tile([P, D], F32, tag="wst2", bufs=2)
            (nc.sync if i % 2 == 0 else nc.scalar).dma_start(
                wst2[:], w2t_d[i * P:(i + 1) * P, :])
            nc.vector.tensor_copy(w2t[i][:], wst2[:])

    # ---- pools for the main loop ----
    xpool = ctx.enter_context(tc.tile_pool(name="xpool", bufs=2))
    xnpool = ctx.enter_context(tc.tile_pool(name="xnpool", bufs=5))
    stat = ctx.enter_context(tc.tile_pool(name="stat", bufs=12))
    scr = ctx.enter_context(tc.tile_pool(name="scr", bufs=2))
    xnt = ctx.enter_context(tc.tile_pool(name="xnt", bufs=6))
    sw = ctx.enter_context(tc.tile_pool(name="sw", bufs=4))
    strips = ctx.enter_context(tc.tile_pool(name="strips", bufs=8))
    diagp = ctx.enter_context(tc.tile_pool(name="diagp", bufs=3))
    vact = ctx.enter_context(tc.tile_pool(name="vact", bufs=32))
    wsbp = ctx.enter_context(tc.tile_pool(name="wsbp", bufs=4))
    outp = ctx.enter_context(tc.tile_pool(name="outp", bufs=3))

    def load_x_panel(b, tp):
        tiles = []
        for tt in range(4):
            t0 = tp * 512 + tt * P
            x_t = xpool.tile([P, D], F32, tag="x", bufs=10, name=f"x_{b}_{tp}_{tt}")
            nc.scalar.dma_start(x_t[:], x_d[b, t0:t0 + P, :])
            tiles.append(x_t)
        return tiles

    def emit_ln_panel(b, tp, x_tiles):
        """LayerNorm + PE-transpose for one 512-token panel -> xnT bf16."""
        means, negvs, stdvs = [], [], []
        for tt in range(4):
            x_t = x_tiles[tt]
            mean = stat.tile([P, 1], F32, tag="mean")
            ex2 = stat.tile([P, 1], F32, tag="ex2")
            xcp = scr.tile([P, D], BF16, tag="xscr")
            nc.scalar.activation(xcp[:], x_t[:], ACTF.Identity,
                                 scale=1.0 / D, accum_out=mean[:])
            xsq = scr.tile([P, D], BF16, tag="xscr")
            nc.scalar.activation(xsq[:], x_t[:], ACTF.Square,
                                 scale=1.0 / np.sqrt(D), accum_out=ex2[:])
            negv = stat.tile([P, 1], F32, tag="negv")
            nc.vector.scalar_tensor_tensor(
                negv[:], mean[:], mean[:], ex2[:],
                op0=ALU.mult, op1=ALU.subtract)
            means.append(mean)
            negvs.append(negv)
        for tt in range(4):
            stdv = stat.tile([P, 1], F32, tag="stdv")
            nc.scalar.activation(stdv[:], negvs[tt][:], ACTF.Sqrt,
                                 scale=-1.0, bias=eps_t[:])
            stdvs.append(stdv)
        xn_tiles = []
        for tt in range(4):
            rstd = stat.tile([P, 1], F32, tag="rstd")
            nc.vector.reciprocal(rstd[:], stdvs[tt][:])
            xn_t = xnpool.tile([P, D], BF16, tag="xn")
            nc.vector.tensor_scalar(
                xn_t[:], x_tiles[tt][:], means[tt][:], rstd[:],
                op0=ALU.subtract, op1=ALU.mult)
            xn_tiles.append(xn_t)
        xnt_p = []
        for j in range(DCH):
            ptr = psum.tile([P, 512], BF16, tag="ps_tr", bufs=2)
            for tt in range(4):
                nc.tensor.transpose(
                    ptr[:, tt * P:(tt + 1) * P],
                    xn_tiles[tt][:, j * P:(j + 1) * P], ident_bf[:])
            xt = xnt.tile([P, 512], BF16, tag="xnt")
            nc.vector.tensor_copy(xt[:], ptr[:])
            xnt_p.append(xt)
        return xnt_p

    xq = {(0, 0): load_x_panel(0, 0)}
    xnt_cache = {(0, 0): emit_ln_panel(0, 0, xq.pop((0, 0)))}
    diags = {}

    def build_diag(b, cb):
        dg = diagp.tile([P, K * P], BF16, tag="diag", name=f"dg_{b}_{cb}")
        for tap in range(K):
            nc.gpsimd.tensor_scalar_mul(
                dg[:, tap * P:(tap + 1) * P], ident_bf[:],
                dw_sb[:, cb * K + tap:cb * K + tap + 1])
        diags[cb] = dg

    for b in range(BPC):
        # ---------- LN + GEMM1 + SwiGLU ----------
        strip = []
        for cb in range(CB):
            s = strips.tile([P, STRIPW], BF16, tag="strip")
            nc.gpsimd.memset(s[:, 0:PADL], 0.0)
            nc.gpsimd.memset(s[:, PADL + T:STRIPW], 0.0)
            strip.append(s)

        for tp in range(TP):
            if tp + 1 < TP:
                xq[(b, tp + 1)] = load_x_panel(b, tp + 1)
            elif b + 1 < BPC:
                xq[(b + 1, 0)] = load_x_panel(b + 1, 0)
            if (b, tp) in xnt_cache:
                xnt_p = xnt_cache.pop((b, tp))
            else:
                xnt_p = emit_ln_panel(b, tp, xq.pop((b, tp)))

            for i in range(CB):
                ps_a = psum.tile([P, 512], F32, tag="ps", bufs=6)
                ps_g = psum.tile([P, 512], F32, tag="ps", bufs=6)
                for j in range(DCH):
                    nc.tensor.matmul(
                        ps_a[:], w1t[j][:, i * P:(i + 1) * P], xnt_p[j][:],
                        start=(j == 0), stop=(j == DCH - 1))
                for j in range(DCH):
                    ii = i + CB
                    nc.tensor.matmul(
                        ps_g[:], w1t[j][:, ii * P:(ii + 1) * P], xnt_p[j][:],
                        start=(j == 0), stop=(j == DCH - 1))
                # u = (a + b1a) * silu(g + b1g)
                s_sb = sw.tile([P, 512], BF16, tag="s_sb")
                nc.scalar.activation(
                    s_sb[:], ps_g[:], ACTF.Silu, bias=b1p[:, i + CB:i + CB + 1])
                a_sb = sw.tile([P, 512], BF16, tag="a_sb")
                nc.scalar.activation(
                    a_sb[:], ps_a[:], ACTF.Identity, bias=b1p[:, i:i + 1])
                nc.vector.tensor_mul(
                    strip[i][:, PADL + tp * 512:PADL + (tp + 1) * 512],
                    a_sb[:], s_sb[:])

        # ---------- depthwise conv + PReLU ----------
        vpan = [[None] * TP for _ in range(CB)]
        build_diag(b, 0)
        for cb in range(CB):
            if cb + 1 < CB:
                build_diag(b, cb + 1)
            dg = diags.pop(cb)
            for tp in range(TP):
                ps_c = psum.tile([P, 512], F32, tag="ps", bufs=6)
                for ti, tap in enumerate(PE_TAPS):
                    off = tp * 512 + tap + 1
                    nc.tensor.matmul(
                        ps_c[:], dg[:, tap * P:(tap + 1) * P],
                        strip[cb][:, off:off + 512],
                        start=(ti == 0), stop=(ti == len(PE_TAPS) - 1))
                # DVE taps: odd k -> even strip offsets (bf16 2x mode)
                td = wsbp.tile([P, 512], BF16, tag="td")
                k0 = DVE_TAPS[0]
                nc.vector.tensor_scalar_mul(
                    td[:], strip[cb][:, tp * 512 + k0 + 1:tp * 512 + k0 + 513],
                    dw_sb[:, cb * K + k0:cb * K + k0 + 1])
                for tap in DVE_TAPS[1:]:
                    off = tp * 512 + tap + 1
                    nc.vector.scalar_tensor_tensor(
                        td[:], strip[cb][:, off:off + 512],
                        dw_sb[:, cb * K + tap:cb * K + tap + 1], td[:],
                        op0=ALU.mult, op1=ALU.add)
                w_sb = wsbp.tile([P, 512], BF16, tag="w_sb")
                nc.vector.tensor_add(w_sb[:], ps_c[:], td[:])
                # v = prelu(w + dwb) with per-channel alpha, in one ACT op
                vt = vact.tile([P, 512], BF16, tag="vact")
                nc.scalar.activation(
                    vt[:], w_sb[:], ACTF.Prelu,
                    bias=dwb_sb[:, cb:cb + 1], alpha=alpha_sb[:, cb:cb + 1])
                vpan[cb][tp] = vt

        # ---------- GEMM2 ----------
        for tp in range(TP):
            for tt in range(4):
                ps_o = psum.tile([P, D], F32, tag="ps", bufs=6)
                nc.tensor.matmul(ps_o[:], ones_bf[:], b2row_bf[:],
                                 start=True, stop=False)
                for cb in range(CB):
                    nc.tensor.matmul(
                        ps_o[:], vpan[cb][tp][:, tt * P:(tt + 1) * P], w2t[cb][:],
                        start=False, stop=(cb == CB - 1))
                o_sb = outp.tile([P, D], F32, tag="o_sb")
                nc.scalar.activation(o_sb[:], ps_o[:], ACTF.Copy)
                t0 = tp * 512 + tt * P
                nc.sync.dma_start(out_d[b, t0:t0 + P, :], o_sb[:])


_NC_CACHE = None


def _get_program():
    global _NC_CACHE
    if _NC_CACHE is None:
        nc = bacc.Bacc("TRN2", target_bir_lowering=False, debug=False)
        with tile.TileContext(nc) as tc, ExitStack() as ctx:
            _build_kernel(ctx, tc)
        nc.compile()
        _NC_CACHE = nc
    return _NC_CACHE


def kernel(x, ln_g, ln_b, w1, b1, dw, dwb, alpha, w2, b2, _trace=False):
    nc = _get_program()
    f32 = np.float32
    x = np.ascontiguousarray(x, f32)
    w1 = np.asarray(w1, f32)
    w2 = np.asarray(w2, f32)
    b1 = np.asarray(b1, f32)
    ln_g = np.asarray(ln_g, f32)
    ln_b = np.asarray(ln_b, f32)
    dw = np.asarray(dw, f32)
    dwb = np.asarray(dwb, f32)
    alpha = np.asarray(alpha, f32)
    b2 = np.asarray(b2, f32)
    # host-side weight prep: fold ln_g into W1, ln_b into b1, pre-transpose
    w1t = np.ascontiguousarray((w1 * ln_g[None, :]).T)            # [D, 2I]
    b1e = b1 + w1 @ ln_b                                          # [2I]
    b1p = np.ascontiguousarray(b1e.reshape(ETILES, P).T)          # [128, 16]
    w2t = np.ascontiguousarray(w2.T)                              # [I, D]
    dwsb = np.ascontiguousarray(
        dw[:, 0, :].reshape(CB, P, K).transpose(1, 0, 2).reshape(P, CB * K))
    dwbp = np.ascontiguousarray(dwb.reshape(CB, P).T)
    alphap = np.ascontiguousarray(alpha.reshape(CB, P).T)
    b2row = np.ascontiguousarray(b2[None, :])
    shared = {
        "w1t": w1t, "w2t": w2t, "b1p": b1p, "dwsb": dwsb,
        "dwbp": dwbp, "alphap": alphap, "b2row": b2row,
    }
    in_maps = [
        {"x": x[c * BPC:(c + 1) * BPC], **shared} for c in range(NCORES)
    ]
    res = run_bass_kernel_spmd(nc, in_maps, core_ids=list(range(NCORES)),
                               trace=_trace)
    out = np.concatenate([res.results[c]["out"] for c in range(NCORES)], axis=0)
    if _trace:
        kernel.last_results = res
    return out


# revision 7
# speedup vs baseline: 1.7253x; 1.7253x over previous
"""Trainium2 Bass kernel for nn_ConvModule: LN -> 1x1 conv (D->2I) -> SwiGLU
-> depthwise conv (K=31) -> PReLU -> 1x1 conv (I->D).

Sharding: data-parallel over batch, 2 batches per core across 8 cores.

v3:
  - Host numpy preprocessing: w1/w2 transposed, ln_g folded into W1, ln_b
    into b1, and the 8 per-channel-block diagonal tap matrices prebuilt as
    bf16 (DMA'd once; no per-batch diag building on any engine).
  - GEMM path all bf16 (fp32 moving operands stream at half PE rate).
  - SwiGLU epilogue: ACT Silu + ACT Identity-with-bias + all-bf16 DVE mul.
  - Conv: PE diag matmuls for 18 taps; 13 odd taps on DVE as an all-bf16
    mult/accumulate chain at 4B-aligned strip offsets; PReLU+dwb in one ACT
    Prelu op. tp-outer loop so GEMM2 interleaves with the conv.
  - DVE ops keep uniform dtypes per op (mixed f32/bf16 operand combos fall
    into a slow DVE path; measured 1.8us vs 0.33us on [128,512]).
"""

import sys

sys.path.insert(0, "/opt/trn_rl_repo")

from contextlib import ExitStack

import numpy as np

import concourse.bacc as bacc
import concourse.tile as tile
from concourse import mybir
from concourse.masks import make_identity
from concourse.bass_utils import run_bass_kernel_spmd

B, T, D, I, K = 16, 2048, 512, 1024, 31
NCORES = 8
BPC = B // NCORES  # batches per core
E = 2 * I  # 2048
TP = T // 512  # time panels per batch (4)
ETILES = E // 128  # 16
CB = I // 128  # channel blocks (8)
DCH = D // 128  # d chunks (4)
PADL = 16  # tap k reads strip offset k+1, so odd k -> 4B-aligned bf16 slice
PADR = 16
STRIPW = PADL + T + PADR  # 2080
NDVE = 8
DVE_TAPS = [2 * i + 1 for i in range(NDVE)]  # 1,3,...,15
PE_TAPS = [k for k in range(K) if k not in DVE_TAPS]  # 23 taps

F32 = mybir.dt.float32
BF16 = mybir.dt.bfloat16
U16 = mybir.dt.uint16
ALU = mybir.AluOpType
ACTF = mybir.ActivationFunctionType
P = 128


def _build_kernel(ctx, tc):
    nc = tc.nc
    x_d = nc.dram_tensor("x", [BPC, T, D], F32, kind="ExternalInput").ap()
    w1t_d = nc.dram_tensor("w1t", [D, E], F32, kind="ExternalInput").ap()
    w2t_d = nc.dram_tensor("w2t", [I, D], F32, kind="ExternalInput").ap()
    b1p_d = nc.dram_tensor("b1p", [P, ETILES], F32, kind="ExternalInput").ap()
    dwsb_d = nc.dram_tensor("dwsb", [P, CB * K], F32, kind="ExternalInput").ap()
    dwb_d = nc.dram_tensor("dwbp", [P, CB], F32, kind="ExternalInput").ap()
    alpha_d = nc.dram_tensor("alphap", [P, CB], F32, kind="ExternalInput").ap()
    b2_d = nc.dram_tensor("b2row", [1, D], F32, kind="ExternalInput").ap()
    # prebuilt bf16 diagonal tap matrices, as uint16 bit patterns
    diag_d = nc.dram_tensor("diagw", [P, CB * K * P], U16,
                            kind="ExternalInput").ap()
    out_d = nc.dram_tensor("out", [BPC, T, D], F32, kind="ExternalOutput").ap()

    const = ctx.enter_context(tc.tile_pool(name="const", bufs=1))
    psum = ctx.enter_context(tc.tile_pool(name="psum", bufs=8, space="PSUM"))

    ident_bf = const.tile([P, P], BF16, tag="ident_bf")
    make_identity(nc, ident_bf[:])

    # ---- parameter tiles ----
    b1p = const.tile([P, ETILES], F32, tag="b1p")
    nc.sync.dma_start(b1p[:], b1p_d)
    alpha_sb = const.tile([P, CB], F32, tag="alpha_sb")
    nc.sync.dma_start(alpha_sb[:], alpha_d)
    dwb_sb = const.tile([P, CB], F32, tag="dwb_sb")
    nc.sync.dma_start(dwb_sb[:], dwb_d)
    dw_sb = const.tile([P, CB * K], F32, tag="dw_sb")
    nc.sync.dma_start(dw_sb[:], dwsb_d)
    eps_t = const.tile([P, 1], F32, tag="eps_t")
    nc.vector.memset(eps_t[:], 1e-5)
    ones_bf = const.tile([1, P], BF16, tag="ones_bf")
    nc.vector.memset(ones_bf[:], 1.0)

    diag = [const.tile([P, K * P], BF16, tag=f"diag{cb}", name=f"diag{cb}")
            for cb in range(CB)]
    for cb in range(CB):
        nc.sync.dma_start(
            diag[cb][:].bitcast(U16),
            diag_d[:, cb * K * P:(cb + 1) * K * P])

    w1t = [const.tile([P, E], BF16, tag=f"w1t{j}", name=f"w1t{j}") for j in range(DCH)]
    w2t = [const.tile([P, D], BF16, tag=f"w2t{i}", name=f"w2t{i}") for i in range(CB)]
    b2row_bf = const.tile([1, D], BF16, tag="b2row_bf")
    with tc.tile_pool(name="setup", bufs=2) as setup:
        b2f = setup.tile([1, D], F32, tag="b2f", bufs=1)
        nc.sync.dma_start(b2f[:], b2_d)
        nc.vector.tensor_copy(b2row_bf[:], b2f[:])
        for j in range(DCH):
            wst = setup.tile([P, E], F32, tag="wst", bufs=2)
            (nc.sync if j % 2 == 0 else nc.scalar).dma_start(
                wst[:], w1t_d[j * P:(j + 1) * P, :])
            nc.vector.tensor_copy(w1t[j][:], wst[:])
        for i in range(CB):
            wst2 = setup.tile([P, D], F32, tag="wst2", bufs=2)
            (nc.sync if i % 2 == 0 else nc.scalar).dma_start(
                wst2[:], w2t_d[i * P:(i + 1) * P, :])
            nc.vector.tensor_copy(w2t[i][:], wst2[:])

    # ---- pools for the main loop ----
    xpool = ctx.enter_context(tc.tile_pool(name="xpool", bufs=2))
    xnpool = ctx.enter_context(tc.tile_pool(name="xnpool", bufs=4))
    xbfpool = ctx.enter_context(tc.tile_pool(name="xbfpool", bufs=5))
    stat = ctx.enter_context(tc.tile_pool(name="stat", bufs=12))
    scr = ctx.enter_context(tc.tile_pool(name="scr", bufs=2))
    xnt = ctx.enter_context(tc.tile_pool(name="xnt", bufs=6))
    sw = ctx.enter_context(tc.tile_pool(name="sw", bufs=4))
    strips = ctx.enter_context(tc.tile_pool(name="strips", bufs=8))
    vact = ctx.enter_context(tc.tile_pool(name="vact", bufs=12))
    wsbp = ctx.enter_context(tc.tile_pool(name="wsbp", bufs=4))
    outp = ctx.enter_context(tc.tile_pool(name="outp", bufs=3))

    def load_x_panel(b, tp):
        tiles = []
        for tt in range(4):
            t0 = tp * 512 + tt * P
            x_t = xpool.tile([P, D], F32, tag="x", bufs=8, name=f"x_{b}_{tp}_{tt}")
            nc.scalar.dma_start(x_t[:], x_d[b, t0:t0 + P, :])
            tiles.append(x_t)
        return tiles

    def emit_ln_panel(b, tp, x_tiles):
        """LayerNorm + PE-transpose for one 512-token panel -> xnT bf16."""
        means, negvs, stdvs = [], [], []
        for tt in range(4):
            x_t = x_tiles[tt]
            mean = stat.tile([P, 1], F32, tag="mean")
            ex2 = stat.tile([P, 1], F32, tag="ex2")
            xcp = scr.tile([P, D], BF16, tag="xscr")
            nc.scalar.activation(xcp[:], x_t[:], ACTF.Identity,
                                 scale=1.0 / D, accum_out=mean[:])
            xsq = scr.tile([P, D], BF16, tag="xscr")
            nc.scalar.activation(xsq[:], x_t[:], ACTF.Square,
                                 scale=1.0 / np.sqrt(D), accum_out=ex2[:])
            negv = stat.tile([P, 1], F32, tag="negv")
            nc.vector.scalar_tensor_tensor(
                negv[:], mean[:], mean[:], ex2[:],
                op0=ALU.mult, op1=ALU.subtract)
            means.append(mean)
            negvs.append(negv)
        for tt in range(4):
            stdv = stat.tile([P, 1], F32, tag="stdv")
            nc.scalar.activation(stdv[:], negvs[tt][:], ACTF.Sqrt,
                                 scale=-1.0, bias=eps_t[:])
            stdvs.append(stdv)
        xn_tiles = []
        for tt in range(4):
            rstd = stat.tile([P, 1], F32, tag="rstd")
            nc.vector.reciprocal(rstd[:], stdvs[tt][:])
            xn_t = xnpool.tile([P, D], F32, tag="xn")
            nc.vector.tensor_scalar(
                xn_t[:], x_tiles[tt][:], means[tt][:], rstd[:],
                op0=ALU.subtract, op1=ALU.mult)
            xn_bf = xbfpool.tile([P, D], BF16, tag="xnbf")
            nc.vector.tensor_copy(xn_bf[:], xn_t[:])
            xn_tiles.append(xn_bf)
        xnt_p = []
        for j in range(DCH):
            ptr = psum.tile([P, 512], BF16, tag="ps_tr", bufs=2)
            for tt in range(4):
                nc.tensor.transpose(
                    ptr[:, tt * P:(tt + 1) * P],
                    xn_tiles[tt][:, j * P:(j + 1) * P], ident_bf[:])
            xt = xnt.tile([P, 512], BF16, tag="xnt")
            nc.vector.tensor_copy(xt[:], ptr[:])
            xnt_p.append(xt)
        return xnt_p

    xq = {(0, 0): load_x_panel(0, 0)}
    xnt_cache = {(0, 0): emit_ln_panel(0, 0, xq.pop((0, 0)))}

    for b in range(BPC):
        # ---------- LN + GEMM1 + SwiGLU ----------
        strip = []
        for cb in range(CB):
            s = strips.tile([P, STRIPW], BF16, tag="strip")
            nc.gpsimd.memset(s[:, 0:PADL], 0.0)
            nc.gpsimd.memset(s[:, PADL + T:STRIPW], 0.0)
            strip.append(s)

        for tp in range(TP):
            if tp + 1 < TP:
                xq[(b, tp + 1)] = load_x_panel(b, tp + 1)
            elif b + 1 < BPC:
                xq[(b + 1, 0)] = load_x_panel(b + 1, 0)
            if (b, tp) in xnt_cache:
                xnt_p = xnt_cache.pop((b, tp))
            else:
                xnt_p = emit_ln_panel(b, tp, xq.pop((b, tp)))

            for i in range(CB):
                ps_a = psum.tile([P, 512], F32, tag="ps", bufs=6)
                ps_g = psum.tile([P, 512], F32, tag="ps", bufs=6)
                for j in range(DCH):
                    nc.tensor.matmul(
                        ps_a[:], w1t[j][:, i * P:(i + 1) * P], xnt_p[j][:],
                        start=(j == 0), stop=(j == DCH - 1))
                for j in range(DCH):
                    ii = i + CB
                    nc.tensor.matmul(
                        ps_g[:], w1t[j][:, ii * P:(ii + 1) * P], xnt_p[j][:],
                        start=(j == 0), stop=(j == DCH - 1))
                # u = (a + b1a) * silu(g + b1g)
                s_sb = sw.tile([P, 512], BF16, tag="s_sb")
                nc.scalar.activation(
                    s_sb[:], ps_g[:], ACTF.Silu, bias=b1p[:, i + CB:i + CB + 1])
                a_sb = sw.tile([P, 512], BF16, tag="a_sb")
                nc.scalar.activation(
                    a_sb[:], ps_a[:], ACTF.Identity, bias=b1p[:, i:i + 1])
                nc.vector.tensor_mul(
                    strip[i][:, PADL + tp * 512:PADL + (tp + 1) * 512],
                    a_sb[:], s_sb[:])

        # ---------- depthwise conv + PReLU + GEMM2, per time panel ----------
        for tp in range(TP):
            vpan = []
            for cb in range(CB):
                ps_c = psum.tile([P, 512], F32, tag="ps", bufs=6)
                for ti, tap in enumerate(PE_TAPS):
                    off = tp * 512 + tap + 1
                    nc.tensor.matmul(
                        ps_c[:], diag[cb][:, tap * P:(tap + 1) * P],
                        strip[cb][:, off:off + 512],
                        start=(ti == 0), stop=(ti == len(PE_TAPS) - 1))
                # DVE taps: seed product on ACT, f32 mult-add chain on DVE
                td = wsbp.tile([P, 512], F32, tag="td")
                k0 = DVE_TAPS[0]
                nc.scalar.activation(
                    td[:], strip[cb][:, tp * 512 + k0 + 1:tp * 512 + k0 + 513],
                    ACTF.Identity, scale=dw_sb[:, cb * K + k0:cb * K + k0 + 1])
                for tap in DVE_TAPS[1:]:
                    off = tp * 512 + tap + 1
                    nc.vector.scalar_tensor_tensor(
                        td[:], strip[cb][:, off:off + 512],
                        dw_sb[:, cb * K + tap:cb * K + tap + 1], td[:],
                        op0=ALU.mult, op1=ALU.add)
                w_sb = wsbp.tile([P, 512], F32, tag="w_sb")
                nc.vector.tensor_add(w_sb[:], ps_c[:], td[:])
                # v = prelu(w + dwb) with per-channel alpha, in one ACT op
                vt = vact.tile([P, 512], BF16, tag="vact")
                nc.scalar.activation(
                    vt[:], w_sb[:], ACTF.Prelu,
                    bias=dwb_sb[:, cb:cb + 1], alpha=alpha_sb[:, cb:cb + 1])
                vpan.append(vt)

            for tt in range(4):
                ps_o = psum.tile([P, D], F32, tag="ps", bufs=6)
                nc.tensor.matmul(ps_o[:], ones_bf[:], b2row_bf[:],
                                 start=True, stop=False)
                for cb in range(CB):
                    nc.tensor.matmul(
                        ps_o[:], vpan[cb][:, tt * P:(tt + 1) * P], w2t[cb][:],
                        start=False, stop=(cb == CB - 1))
                o_sb = outp.tile([P, D], F32, tag="o_sb")
                nc.scalar.activation(o_sb[:], ps_o[:], ACTF.Copy)
                t0 = tp * 512 + tt * P
                nc.sync.dma_start(out_d[b, t0:t0 + P, :], o_sb[:])


_NC_CACHE = None


def _get_program():
    global _NC_CACHE
    if _NC_CACHE is None:
        nc = bacc.Bacc("TRN2", target_bir_lowering=False, debug=False)
        with tile.TileContext(nc) as tc, ExitStack() as ctx:
            _build_kernel(ctx, tc)
        nc.compile()
        _NC_CACHE = nc
    return _NC_CACHE


def _bf16_bits(a):
    """RNE float32 -> bf16 bit pattern, as uint16."""
    u = np.ascontiguousarray(a, np.float32).view(np.uint32)
    rounded = u + 0x7FFF + ((u >> 16) & 1)
    return (rounded >> 16).astype(np.uint16)


def host_prep(ln_g, ln_b, w1, b1, dw, dwb, alpha, w2, b2):
    f32 = np.float32
    w1 = np.asarray(w1, f32)
    w2 = np.asarray(w2, f32)
    b1 = np.asarray(b1, f32)
    ln_g = np.asarray(ln_g, f32)
    ln_b = np.asarray(ln_b, f32)
    dw = np.asarray(dw, f32)
    dwb = np.asarray(dwb, f32)
    alpha = np.asarray(alpha, f32)
    b2 = np.asarray(b2, f32)
    w1t = np.ascontiguousarray((w1 * ln_g[None, :]).T)            # [D, 2I]
    b1e = b1 + w1 @ ln_b                                          # [2I]
    b1p = np.ascontiguousarray(b1e.reshape(ETILES, P).T)          # [128, 16]
    w2t = np.ascontiguousarray(w2.T)                              # [I, D]
    dwf = dw[:, 0, :].reshape(CB, P, K)
    dwsb = np.ascontiguousarray(dwf.transpose(1, 0, 2).reshape(P, CB * K))
    # diag[p, cb, k, q] = (p==q) * dw[cb*128+p, k], bf16 bits
    dia = np.einsum('pq,cpk->pckq', np.eye(P, dtype=f32), dwf)
    diagw = np.ascontiguousarray(_bf16_bits(dia).reshape(P, CB * K * P))
    return {
        "w1t": w1t, "w2t": w2t, "b1p": b1p, "dwsb": dwsb,
        "dwbp": np.ascontiguousarray(dwb.reshape(CB, P).T),
        "alphap": np.ascontiguousarray(alpha.reshape(CB, P).T),
        "b2row": np.ascontiguousarray(b2[None, :]),
        "diagw": diagw,
    }


def kernel(x, ln_g, ln_b, w1, b1, dw, dwb, alpha, w2, b2, _trace=False):
    nc = _get_program()
    x = np.ascontiguousarray(x, np.float32)
    shared = host_prep(ln_g, ln_b, w1, b1, dw, dwb, alpha, w2, b2)
    in_maps = [
        {"x": x[c * BPC:(c + 1) * BPC], **shared} for c in range(NCORES)
    ]
    res = run_bass_kernel_spmd(nc, in_maps, core_ids=list(range(NCORES)),
                               trace=_trace)
    out = np.concatenate([res.results[c]["out"] for c in range(NCORES)], axis=0)
    if _trace:
        kernel.last_results = res
    return out


# revision 8
# speedup vs baseline: 1.7542x; 1.0168x over previous
"""Trainium2 Bass kernel for nn_ConvModule: LN -> 1x1 conv (D->2I) -> SwiGLU
-> depthwise conv (K=31) -> PReLU -> 1x1 conv (I->D).

Sharding: data-parallel over batch, 2 batches per core across 8 cores.

v3:
  - Host numpy preprocessing: w1/w2 transposed, ln_g folded into W1, ln_b
    into b1, and the 8 per-channel-block diagonal tap matrices prebuilt as
    bf16 (DMA'd once; no per-batch diag building on any engine).
  - GEMM path all bf16 (fp32 moving operands stream at half PE rate).
  - SwiGLU epilogue: ACT Silu + ACT Identity-with-bias + all-bf16 DVE mul.
  - Conv: PE diag matmuls for 18 taps; 13 odd taps on DVE as an all-bf16
    mult/accumulate chain at 4B-aligned strip offsets; PReLU+dwb in one ACT
    Prelu op. tp-outer loop so GEMM2 interleaves with the conv.
  - DVE ops keep uniform dtypes per op (mixed f32/bf16 operand combos fall
    into a slow DVE path; measured 1.8us vs 0.33us on [128,512]).
"""

import sys

sys.path.insert(0, "/opt/trn_rl_repo")

from contextlib import ExitStack

import numpy as np

import concourse.bacc as bacc
import concourse.tile as tile
from concourse import mybir
from concourse.masks import make_identity
from concourse.bass_utils import run_bass_kernel_spmd

B, T, D, I, K = 16, 2048, 512, 1024, 31
NCORES = 8
BPC = B // NCORES  # batches per core
E = 2 * I  # 2048
TP = T // 512  # time panels per batch (4)
ETILES = E // 128  # 16
CB = I // 128  # channel blocks (8)
DCH = D // 128  # d chunks (4)
PADL = 16  # tap k reads strip offset k+1, so odd k -> 4B-aligned bf16 slice
PADR = 16
STRIPW = PADL + T + PADR  # 2080
NDVE = 9
DVE_TAPS = [2 * i + 1 for i in range(NDVE)]  # 1,3,...,17
PE_TAPS = [k for k in range(K) if k not in DVE_TAPS]  # 22 taps

F32 = mybir.dt.float32
BF16 = mybir.dt.bfloat16
U16 = mybir.dt.uint16
ALU = mybir.AluOpType
ACTF = mybir.ActivationFunctionType
P = 128


def _build_kernel(ctx, tc):
    nc = tc.nc
    x_d = nc.dram_tensor("x", [BPC, T, D], F32, kind="ExternalInput").ap()
    w1t_d = nc.dram_tensor("w1t", [D, E], F32, kind="ExternalInput").ap()
    w2t_d = nc.dram_tensor("w2t", [I, D], F32, kind="ExternalInput").ap()
    b1p_d = nc.dram_tensor("b1p", [P, ETILES], F32, kind="ExternalInput").ap()
    dwsb_d = nc.dram_tensor("dwsb", [P, CB * K], F32, kind="ExternalInput").ap()
    dwb_d = nc.dram_tensor("dwbp", [P, CB], F32, kind="ExternalInput").ap()
    alpha_d = nc.dram_tensor("alphap", [P, CB], F32, kind="ExternalInput").ap()
    b2_d = nc.dram_tensor("b2row", [1, D], F32, kind="ExternalInput").ap()
    # prebuilt bf16 diagonal tap matrices, as uint16 bit patterns
    diag_d = nc.dram_tensor("diagw", [P, CB * K * P], U16,
                            kind="ExternalInput").ap()
    out_d = nc.dram_tensor("out", [BPC, T, D], F32, kind="ExternalOutput").ap()

    const = ctx.enter_context(tc.tile_pool(name="const", bufs=1))
    psum = ctx.enter_context(tc.tile_pool(name="psum", bufs=8, space="PSUM"))

    ident_bf = const.tile([P, P], BF16, tag="ident_bf")
    make_identity(nc, ident_bf[:])

    # ---- parameter tiles ----
    b1p = const.tile([P, ETILES], F32, tag="b1p")
    nc.sync.dma_start(b1p[:], b1p_d)
    alpha_sb = const.tile([P, CB], F32, tag="alpha_sb")
    nc.sync.dma_start(alpha_sb[:], alpha_d)
    dwb_sb = const.tile([P, CB], F32, tag="dwb_sb")
    nc.sync.dma_start(dwb_sb[:], dwb_d)
    dw_sb = const.tile([P, CB * K], F32, tag="dw_sb")
    nc.sync.dma_start(dw_sb[:], dwsb_d)
    eps_t = const.tile([P, 1], F32, tag="eps_t")
    nc.vector.memset(eps_t[:], 1e-5)
    ones_bf = const.tile([1, P], BF16, tag="ones_bf")
    nc.vector.memset(ones_bf[:], 1.0)

    diag = [const.tile([P, K * P], BF16, tag=f"diag{cb}", name=f"diag{cb}")
            for cb in range(CB)]

    def load_diags():
        for cb in range(CB):
            nc.sync.dma_start(
                diag[cb][:].bitcast(U16),
                diag_d[:, cb * K * P:(cb + 1) * K * P])

    w1t = [const.tile([P, E], BF16, tag=f"w1t{j}", name=f"w1t{j}") for j in range(DCH)]
    w2t = [const.tile([P, D], BF16, tag=f"w2t{i}", name=f"w2t{i}") for i in range(CB)]
    b2row_bf = const.tile([1, D], BF16, tag="b2row_bf")
    with tc.tile_pool(name="setup", bufs=2) as setup:
        b2f = setup.tile([1, D], F32, tag="b2f", bufs=1)
        nc.sync.dma_start(b2f[:], b2_d)
        nc.vector.tensor_copy(b2row_bf[:], b2f[:])
        for j in range(DCH):
            wst = setup.tile([P, E], F32, tag="wst", bufs=2)
            nc.sync.dma_start(wst[:], w1t_d[j * P:(j + 1) * P, :])
            nc.vector.tensor_copy(w1t[j][:], wst[:])
        for i in range(CB):
            wst2 = setup.tile([P, D], F32, tag="wst2", bufs=2)
            nc.sync.dma_start(wst2[:], w2t_d[i * P:(i + 1) * P, :])
            nc.vector.tensor_copy(w2t[i][:], wst2[:])

    # ---- pools for the main loop ----
    xpool = ctx.enter_context(tc.tile_pool(name="xpool", bufs=2))
    xbfpool = ctx.enter_context(tc.tile_pool(name="xbfpool", bufs=5))
    stat = ctx.enter_context(tc.tile_pool(name="stat", bufs=12))
    scr = ctx.enter_context(tc.tile_pool(name="scr", bufs=2))
    xnt = ctx.enter_context(tc.tile_pool(name="xnt", bufs=6))
    sw = ctx.enter_context(tc.tile_pool(name="sw", bufs=4))
    strips = ctx.enter_context(tc.tile_pool(name="strips", bufs=8))
    vact = ctx.enter_context(tc.tile_pool(name="vact", bufs=12))
    wsbp = ctx.enter_context(tc.tile_pool(name="wsbp", bufs=4))
    outp = ctx.enter_context(tc.tile_pool(name="outp", bufs=3))

    def load_x_panel(b, tp):
        tiles = []
        for tt in range(4):
            t0 = tp * 512 + tt * P
            x_t = xpool.tile([P, D], F32, tag="x", bufs=8, name=f"x_{b}_{tp}_{tt}")
            nc.scalar.dma_start(x_t[:], x_d[b, t0:t0 + P, :])
            tiles.append(x_t)
        return tiles

    def emit_ln_panel(b, tp, x_tiles):
        """LayerNorm + PE-transpose for one 512-token panel -> xnT bf16."""
        means, negvs, stdvs = [], [], []
        for tt in range(4):
            x_t = x_tiles[tt]
            mean = stat.tile([P, 1], F32, tag="mean")
            ex2 = stat.tile([P, 1], F32, tag="ex2")
            xcp = scr.tile([P, D], BF16, tag="xscr")
            nc.scalar.activation(xcp[:], x_t[:], ACTF.Identity,
                                 scale=1.0 / D, accum_out=mean[:])
            xsq = scr.tile([P, D], BF16, tag="xscr")
            nc.scalar.activation(xsq[:], x_t[:], ACTF.Square,
                                 scale=1.0 / np.sqrt(D), accum_out=ex2[:])
            negv = stat.tile([P, 1], F32, tag="negv")
            nc.vector.scalar_tensor_tensor(
                negv[:], mean[:], mean[:], ex2[:],
                op0=ALU.mult, op1=ALU.subtract)
            means.append(mean)
            negvs.append(negv)
        for tt in range(4):
            stdv = stat.tile([P, 1], F32, tag="stdv")
            nc.scalar.activation(stdv[:], negvs[tt][:], ACTF.Sqrt,
                                 scale=-1.0, bias=eps_t[:])
            stdvs.append(stdv)
        xn_tiles = []
        for tt in range(4):
            rstd = stat.tile([P, 1], F32, tag="rstd")
            nc.vector.reciprocal(rstd[:], stdvs[tt][:])
            nbias = stat.tile([P, 1], F32, tag="nbias")
            nc.vector.scalar_tensor_tensor(
                nbias[:], means[tt][:], -1.0, rstd[:],
                op0=ALU.mult, op1=ALU.mult)
            xn_bf = xbfpool.tile([P, D], BF16, tag="xnbf")
            nc.scalar.activation(xn_bf[:], x_tiles[tt][:], ACTF.Identity,
                                 bias=nbias[:], scale=rstd[:])
            xn_tiles.append(xn_bf)
        xnt_p = []
        for j in range(DCH):
            ptr = psum.tile([P, 512], BF16, tag="ps_tr", bufs=2)
            for tt in range(4):
                nc.tensor.transpose(
                    ptr[:, tt * P:(tt + 1) * P],
                    xn_tiles[tt][:, j * P:(j + 1) * P], ident_bf[:])
            xt = xnt.tile([P, 512], BF16, tag="xnt")
            nc.vector.tensor_copy(xt[:], ptr[:])
            xnt_p.append(xt)
        return xnt_p

    xq = {(0, 0): load_x_panel(0, 0)}
    xnt_cache = {(0, 0): emit_ln_panel(0, 0, xq.pop((0, 0)))}

    for b in range(BPC):
        # ---------- LN + GEMM1 + SwiGLU ----------
        strip = []
        for cb in range(CB):
            s = strips.tile([P, STRIPW], BF16, tag="strip")
            nc.gpsimd.memset(s[:, 0:PADL], 0.0)
            nc.gpsimd.memset(s[:, PADL + T:STRIPW], 0.0)
            strip.append(s)

        for tp in range(TP):
            if tp + 1 < TP:
                xq[(b, tp + 1)] = load_x_panel(b, tp + 1)
            elif b + 1 < BPC:
                xq[(b + 1, 0)] = load_x_panel(b + 1, 0)
            if (b, tp) in xnt_cache:
                xnt_p = xnt_cache.pop((b, tp))
            else:
                xnt_p = emit_ln_panel(b, tp, xq.pop((b, tp)))

            for i in range(CB):
                ps_a = psum.tile([P, 512], F32, tag="ps", bufs=6)
                ps_g = psum.tile([P, 512], F32, tag="ps", bufs=6)
                for j in range(DCH):
                    nc.tensor.matmul(
                        ps_a[:], w1t[j][:, i * P:(i + 1) * P], xnt_p[j][:],
                        start=(j == 0), stop=(j == DCH - 1))
                for j in range(DCH):
                    ii = i + CB
                    nc.tensor.matmul(
                        ps_g[:], w1t[j][:, ii * P:(ii + 1) * P], xnt_p[j][:],
                        start=(j == 0), stop=(j == DCH - 1))
                # u = (a + b1a) * silu(g + b1g)
                s_sb = sw.tile([P, 512], BF16, tag="s_sb")
                nc.scalar.activation(
                    s_sb[:], ps_g[:], ACTF.Silu, bias=b1p[:, i + CB:i + CB + 1])
                a_sb = sw.tile([P, 512], BF16, tag="a_sb")
                nc.scalar.activation(
                    a_sb[:], ps_a[:], ACTF.Identity, bias=b1p[:, i:i + 1])
                nc.vector.tensor_mul(
                    strip[i][:, PADL + tp * 512:PADL + (tp + 1) * 512],
                    a_sb[:], s_sb[:])

        if b == 0:
            load_diags()

        # ---------- depthwise conv + PReLU + GEMM2, per time panel ----------
        for tp in range(TP):
            vpan = []
            for cb in range(CB):
                ps_c = psum.tile([P, 512], F32, tag="ps", bufs=6)
                for ti, tap in enumerate(PE_TAPS):
                    off = tp * 512 + tap + 1
                    nc.tensor.matmul(
                        ps_c[:], diag[cb][:, tap * P:(tap + 1) * P],
                        strip[cb][:, off:off + 512],
                        start=(ti == 0), stop=(ti == len(PE_TAPS) - 1))
                # DVE taps: seed product on ACT, f32 mult-add chain on DVE
                td = wsbp.tile([P, 512], F32, tag="td")
                k0 = DVE_TAPS[0]
                nc.scalar.activation(
                    td[:], strip[cb][:, tp * 512 + k0 + 1:tp * 512 + k0 + 513],
                    ACTF.Identity, scale=dw_sb[:, cb * K + k0:cb * K + k0 + 1])
                for tap in DVE_TAPS[1:]:
                    off = tp * 512 + tap + 1
                    nc.vector.scalar_tensor_tensor(
                        td[:], strip[cb][:, off:off + 512],
                        dw_sb[:, cb * K + tap:cb * K + tap + 1], td[:],
                        op0=ALU.mult, op1=ALU.add)
                w_sb = wsbp.tile([P, 512], F32, tag="w_sb")
                nc.vector.tensor_add(w_sb[:], ps_c[:], td[:])
                # v = prelu(w + dwb) with per-channel alpha, in one ACT op
                vt = vact.tile([P, 512], BF16, tag="vact")
                nc.scalar.activation(
                    vt[:], w_sb[:], ACTF.Prelu,
                    bias=dwb_sb[:, cb:cb + 1], alpha=alpha_sb[:, cb:cb + 1])
                vpan.append(vt)

            for tt in range(4):
                ps_o = psum.tile([P, D], F32, tag="ps", bufs=6)
                nc.tensor.matmul(ps_o[:], ones_bf[:], b2row_bf[:],
                                 start=True, stop=False)
                for cb in range(CB):
                    nc.tensor.matmul(
                        ps_o[:], vpan[cb][:, tt * P:(tt + 1) * P], w2t[cb][:],
                        start=False, stop=(cb == CB - 1))
                o_sb = outp.tile([P, D], F32, tag="o_sb")
                nc.scalar.activation(o_sb[:], ps_o[:], ACTF.Copy)
                t0 = tp * 512 + tt * P
                nc.sync.dma_start(out_d[b, t0:t0 + P, :], o_sb[:])


_NC_CACHE = None


def _get_program():
    global _NC_CACHE
    if _NC_CACHE is None:
        nc = bacc.Bacc("TRN2", target_bir_lowering=False, debug=False)
        with tile.TileContext(nc) as tc, ExitStack() as ctx:
            _build_kernel(ctx, tc)
        nc.compile()
        _NC_CACHE = nc
    return _NC_CACHE


def _bf16_bits(a):
    """RNE float32 -> bf16 bit pattern, as uint16."""
    u = np.ascontiguousarray(a, np.float32).view(np.uint32)
    rounded = u + 0x7FFF + ((u >> 16) & 1)
    return (rounded >> 16).astype(np.uint16)


def host_prep(ln_g, ln_b, w1, b1, dw, dwb, alpha, w2, b2):
    f32 = np.float32
    w1 = np.asarray(w1, f32)
    w2 = np.asarray(w2, f32)
    b1 = np.asarray(b1, f32)
    ln_g = np.asarray(ln_g, f32)
    ln_b = np.asarray(ln_b, f32)
    dw = np.asarray(dw, f32)
    dwb = np.asarray(dwb, f32)
    alpha = np.asarray(alpha, f32)
    b2 = np.asarray(b2, f32)
    w1t = np.ascontiguousarray((w1 * ln_g[None, :]).T)            # [D, 2I]
    b1e = b1 + w1 @ ln_b                                          # [2I]
    b1p = np.ascontiguousarray(b1e.reshape(ETILES, P).T)          # [128, 16]
    w2t = np.ascontiguousarray(w2.T)                              # [I, D]
    dwf = dw[:, 0, :].reshape(CB, P, K)
    dwsb = np.ascontiguousarray(dwf.transpose(1, 0, 2).reshape(P, CB * K))
    # diag[p, cb, k, q] = (p==q) * dw[cb*128+p, k], bf16 bits
    dia = np.einsum('pq,cpk->pckq', np.eye(P, dtype=f32), dwf)
    diagw = np.ascontiguousarray(_bf16_bits(dia).reshape(P, CB * K * P))
    return {
        "w1t": w1t, "w2t": w2t, "b1p": b1p, "dwsb": dwsb,
        "dwbp": np.ascontiguousarray(dwb.reshape(CB, P).T),
        "alphap": np.ascontiguousarray(alpha.reshape(CB, P).T),
        "b2row": np.ascontiguousarray(b2[None, :]),
        "diagw": diagw,
    }


def kernel(x, ln_g, ln_b, w1, b1, dw, dwb, alpha, w2, b2, _trace=False):
    nc = _get_program()
    x = np.ascontiguousarray(x, np.float32)
    shared = host_prep(ln_g, ln_b, w1, b1, dw, dwb, alpha, w2, b2)
    in_maps = [
        {"x": x[c * BPC:(c + 1) * BPC], **shared} for c in range(NCORES)
    ]
    res = run_bass_kernel_spmd(nc, in_maps, core_ids=list(range(NCORES)),
                               trace=_trace)
    out = np.concatenate([res.results[c]["out"] for c in range(NCORES)], axis=0)
    if _trace:
        kernel.last_results = res
    return out


# revision 9
# speedup vs baseline: 1.8088x; 1.0311x over previous
"""Trainium2 Bass kernel for nn_ConvModule: LN -> 1x1 conv (D->2I) -> SwiGLU
-> depthwise conv (K=31) -> PReLU -> 1x1 conv (I->D).

Sharding: data-parallel over batch, 2 batches per core across 8 cores.

v3:
  - Host numpy preprocessing: w1/w2 transposed, ln_g folded into W1, ln_b
    into b1, and the 8 per-channel-block diagonal tap matrices prebuilt as
    bf16 (DMA'd once; no per-batch diag building on any engine).
  - GEMM path all bf16 (fp32 moving operands stream at half PE rate).
  - SwiGLU epilogue: ACT Silu + ACT Identity-with-bias + all-bf16 DVE mul.
  - Conv: PE diag matmuls for 18 taps; 13 odd taps on DVE as an all-bf16
    mult/accumulate chain at 4B-aligned strip offsets; PReLU+dwb in one ACT
    Prelu op. tp-outer loop so GEMM2 interleaves with the conv.
  - DVE ops keep uniform dtypes per op (mixed f32/bf16 operand combos fall
    into a slow DVE path; measured 1.8us vs 0.33us on [128,512]).
"""

import sys

sys.path.insert(0, "/opt/trn_rl_repo")

from contextlib import ExitStack

import numpy as np

import concourse.bacc as bacc
import concourse.tile as tile
from concourse import mybir
from concourse.masks import make_identity
from concourse.bass_utils import run_bass_kernel_spmd

B, T, D, I, K = 16, 2048, 512, 1024, 31
NCORES = 8
BPC = B // NCORES  # batches per core
E = 2 * I  # 2048
TP = T // 512  # time panels per batch (4)
ETILES = E // 128  # 16
CB = I // 128  # channel blocks (8)
DCH = D // 128  # d chunks (4)
PADL = 16  # tap k reads strip offset k+1, so odd k -> 4B-aligned bf16 slice
PADR = 16
STRIPW = PADL + T + PADR  # 2080
NDVE = 9
DVE_TAPS = [2 * i + 1 for i in range(NDVE)]  # 1,3,...,17
PE_TAPS = [k for k in range(K) if k not in DVE_TAPS]  # 22 taps

F32 = mybir.dt.float32
BF16 = mybir.dt.bfloat16
U16 = mybir.dt.uint16
ALU = mybir.AluOpType
ACTF = mybir.ActivationFunctionType
P = 128


def _build_kernel(ctx, tc):
    nc = tc.nc
    x_d = nc.dram_tensor("x", [BPC, T, D], F32, kind="ExternalInput").ap()
    w1t_d = nc.dram_tensor("w1t", [D, E], F32, kind="ExternalInput").ap()
    w2t_d = nc.dram_tensor("w2t", [I, D], F32, kind="ExternalInput").ap()
    b1p_d = nc.dram_tensor("b1p", [P, ETILES], F32, kind="ExternalInput").ap()
    dwsb_d = nc.dram_tensor("dwsb", [P, CB * K], F32, kind="ExternalInput").ap()
    dwb_d = nc.dram_tensor("dwbp", [P, CB], F32, kind="ExternalInput").ap()
    alpha_d = nc.dram_tensor("alphap", [P, CB], F32, kind="ExternalInput").ap()
    b2_d = nc.dram_tensor("b2row", [1, D], F32, kind="ExternalInput").ap()
    # prebuilt bf16 diagonal tap matrices, as uint16 bit patterns
    diag_d = nc.dram_tensor("diagw", [P, CB * K * P], U16,
                            kind="ExternalInput").ap()
    out_d = nc.dram_tensor("out", [BPC, T, D], F32, kind="ExternalOutput").ap()

    const = ctx.enter_context(tc.tile_pool(name="const", bufs=1))
    psum = ctx.enter_context(tc.tile_pool(name="psum", bufs=8, space="PSUM"))

    ident_bf = const.tile([P, P], BF16, tag="ident_bf")
    make_identity(nc, ident_bf[:])

    # ---- parameter tiles ----
    b1p = const.tile([P, ETILES], F32, tag="b1p")
    nc.sync.dma_start(b1p[:], b1p_d)
    alpha_sb = const.tile([P, CB], F32, tag="alpha_sb")
    nc.sync.dma_start(alpha_sb[:], alpha_d)
    dwb_sb = const.tile([P, CB], F32, tag="dwb_sb")
    nc.sync.dma_start(dwb_sb[:], dwb_d)
    dw_sb = const.tile([P, CB * K], F32, tag="dw_sb")
    nc.sync.dma_start(dw_sb[:], dwsb_d)
    eps_t = const.tile([P, 1], F32, tag="eps_t")
    nc.vector.memset(eps_t[:], 1e-5)
    ones_bf = const.tile([1, P], BF16, tag="ones_bf")
    nc.vector.memset(ones_bf[:], 1.0)

    diag = [const.tile([P, K * P], BF16, tag=f"diag{cb}", name=f"diag{cb}")
            for cb in range(CB)]

    def load_diags():
        for cb in range(CB):
            nc.sync.dma_start(
                diag[cb][:].bitcast(U16),
                diag_d[:, cb * K * P:(cb + 1) * K * P])

    w1t = [const.tile([P, E], BF16, tag=f"w1t{j}", name=f"w1t{j}") for j in range(DCH)]
    w2t = [const.tile([P, D], BF16, tag=f"w2t{i}", name=f"w2t{i}") for i in range(CB)]
    b2row_bf = const.tile([1, D], BF16, tag="b2row_bf")
    with tc.tile_pool(name="setup", bufs=2) as setup:
        b2f = setup.tile([1, D], F32, tag="b2f", bufs=1)
        nc.sync.dma_start(b2f[:], b2_d)
        nc.vector.tensor_copy(b2row_bf[:], b2f[:])
        for j in range(DCH):
            wst = setup.tile([P, E], F32, tag="wst", bufs=2)
            nc.sync.dma_start(wst[:], w1t_d[j * P:(j + 1) * P, :])
            nc.vector.tensor_copy(w1t[j][:], wst[:])
        for i in range(CB):
            wst2 = setup.tile([P, D], F32, tag="wst2", bufs=2)
            nc.sync.dma_start(wst2[:], w2t_d[i * P:(i + 1) * P, :])
            nc.vector.tensor_copy(w2t[i][:], wst2[:])

    # ---- pools for the main loop ----
    xpool = ctx.enter_context(tc.tile_pool(name="xpool", bufs=2))
    xnpool = ctx.enter_context(tc.tile_pool(name="xnpool", bufs=4))
    xbfpool = ctx.enter_context(tc.tile_pool(name="xbfpool", bufs=5))
    stat = ctx.enter_context(tc.tile_pool(name="stat", bufs=12))
    scr = ctx.enter_context(tc.tile_pool(name="scr", bufs=2))
    xnt = ctx.enter_context(tc.tile_pool(name="xnt", bufs=6))
    sw = ctx.enter_context(tc.tile_pool(name="sw", bufs=4))
    strips = ctx.enter_context(tc.tile_pool(name="strips", bufs=8))
    vact = ctx.enter_context(tc.tile_pool(name="vact", bufs=12))
    wsbp = ctx.enter_context(tc.tile_pool(name="wsbp", bufs=4))
    outp = ctx.enter_context(tc.tile_pool(name="outp", bufs=3))

    def load_x_panel(b, tp):
        tiles = []
        for tt in range(4):
            t0 = tp * 512 + tt * P
            x_t = xpool.tile([P, D], F32, tag="x", bufs=8, name=f"x_{b}_{tp}_{tt}")
            nc.scalar.dma_start(x_t[:], x_d[b, t0:t0 + P, :])
            tiles.append(x_t)
        return tiles

    def emit_ln_panel(b, tp, x_tiles):
        """LayerNorm + PE-transpose for one 512-token panel -> xnT bf16."""
        means, negvs, stdvs = [], [], []
        for tt in range(4):
            x_t = x_tiles[tt]
            mean = stat.tile([P, 1], F32, tag="mean")
            ex2 = stat.tile([P, 1], F32, tag="ex2")
            xcp = scr.tile([P, D], BF16, tag="xscr")
            nc.scalar.activation(xcp[:], x_t[:], ACTF.Identity,
                                 scale=1.0 / D, accum_out=mean[:])
            xsq = scr.tile([P, D], BF16, tag="xscr")
            nc.scalar.activation(xsq[:], x_t[:], ACTF.Square,
                                 scale=1.0 / np.sqrt(D), accum_out=ex2[:])
            negv = stat.tile([P, 1], F32, tag="negv")
            nc.vector.scalar_tensor_tensor(
                negv[:], mean[:], mean[:], ex2[:],
                op0=ALU.mult, op1=ALU.subtract)
            means.append(mean)
            negvs.append(negv)
        for tt in range(4):
            stdv = stat.tile([P, 1], F32, tag="stdv")
            nc.scalar.activation(stdv[:], negvs[tt][:], ACTF.Sqrt,
                                 scale=-1.0, bias=eps_t[:])
            stdvs.append(stdv)
        xn_tiles = []
        for tt in range(4):
            rstd = stat.tile([P, 1], F32, tag="rstd")
            nc.vector.reciprocal(rstd[:], stdvs[tt][:])
            xn_t = xnpool.tile([P, D], F32, tag="xn")
            nc.vector.tensor_scalar(
                xn_t[:], x_tiles[tt][:], means[tt][:], rstd[:],
                op0=ALU.subtract, op1=ALU.mult)
            xn_bf = xbfpool.tile([P, D], BF16, tag="xnbf")
            nc.vector.tensor_copy(xn_bf[:], xn_t[:])
            xn_tiles.append(xn_bf)
        xnt_p = []
        for j in range(DCH):
            ptr = psum.tile([P, 512], BF16, tag="ps_tr", bufs=2)
            for tt in range(4):
                nc.tensor.transpose(
                    ptr[:, tt * P:(tt + 1) * P],
                    xn_tiles[tt][:, j * P:(j + 1) * P], ident_bf[:])
            xt = xnt.tile([P, 512], BF16, tag="xnt")
            nc.vector.tensor_copy(xt[:], ptr[:])
            xnt_p.append(xt)
        return xnt_p

    xq = {(0, 0): load_x_panel(0, 0)}
    xnt_cache = {(0, 0): emit_ln_panel(0, 0, xq.pop((0, 0)))}

    for b in range(BPC):
        # ---------- LN + GEMM1 + SwiGLU ----------
        strip = []
        for cb in range(CB):
            s = strips.tile([P, STRIPW], BF16, tag="strip")
            nc.gpsimd.memset(s[:, 0:PADL], 0.0)
            nc.gpsimd.memset(s[:, PADL + T:STRIPW], 0.0)
            strip.append(s)

        for tp in range(TP):
            if tp + 1 < TP:
                xq[(b, tp + 1)] = load_x_panel(b, tp + 1)
            elif b + 1 < BPC:
                xq[(b + 1, 0)] = load_x_panel(b + 1, 0)
            if (b, tp) in xnt_cache:
                xnt_p = xnt_cache.pop((b, tp))
            else:
                xnt_p = emit_ln_panel(b, tp, xq.pop((b, tp)))

            for i in range(CB):
                ps_a = psum.tile([P, 512], F32, tag="ps", bufs=6)
                ps_g = psum.tile([P, 512], F32, tag="ps", bufs=6)
                for j in range(DCH):
                    nc.tensor.matmul(
                        ps_a[:], w1t[j][:, i * P:(i + 1) * P], xnt_p[j][:],
                        start=(j == 0), stop=(j == DCH - 1))
                for j in range(DCH):
                    ii = i + CB
                    nc.tensor.matmul(
                        ps_g[:], w1t[j][:, ii * P:(ii + 1) * P], xnt_p[j][:],
                        start=(j == 0), stop=(j == DCH - 1))
                # u = (a + b1a) * silu(g + b1g)
                s_sb = sw.tile([P, 512], BF16, tag="s_sb")
                nc.scalar.activation(
                    s_sb[:], ps_g[:], ACTF.Silu, bias=b1p[:, i + CB:i + CB + 1])
                nc.vector.scalar_tensor_tensor(
                    strip[i][:, PADL + tp * 512:PADL + (tp + 1) * 512],
                    ps_a[:], b1p[:, i:i + 1], s_sb[:],
                    op0=ALU.add, op1=ALU.mult)

        if b == 0:
            load_diags()

        # ---------- depthwise conv + PReLU + GEMM2, per time panel ----------
        for tp in range(TP):
            vpan = []
            for cb in range(CB):
                ps_c = psum.tile([P, 512], F32, tag="ps", bufs=6)
                for ti, tap in enumerate(PE_TAPS):
                    off = tp * 512 + tap + 1
                    nc.tensor.matmul(
                        ps_c[:], diag[cb][:, tap * P:(tap + 1) * P],
                        strip[cb][:, off:off + 512],
                        start=(ti == 0), stop=(ti == len(PE_TAPS) - 1))
                # DVE taps: seed product on ACT, f32 mult-add chain on DVE
                td = wsbp.tile([P, 512], F32, tag="td")
                k0 = DVE_TAPS[0]
                nc.scalar.activation(
                    td[:], strip[cb][:, tp * 512 + k0 + 1:tp * 512 + k0 + 513],
                    ACTF.Identity, scale=dw_sb[:, cb * K + k0:cb * K + k0 + 1])
                for tap in DVE_TAPS[1:]:
                    off = tp * 512 + tap + 1
                    nc.vector.scalar_tensor_tensor(
                        td[:], strip[cb][:, off:off + 512],
                        dw_sb[:, cb * K + tap:cb * K + tap + 1], td[:],
                        op0=ALU.mult, op1=ALU.add)
                w_sb = wsbp.tile([P, 512], F32, tag="w_sb")
                nc.vector.tensor_add(w_sb[:], ps_c[:], td[:])
                # v = prelu(w + dwb) with per-channel alpha, in one ACT op
                vt = vact.tile([P, 512], BF16, tag="vact")
                nc.scalar.activation(
                    vt[:], w_sb[:], ACTF.Prelu,
                    bias=dwb_sb[:, cb:cb + 1], alpha=alpha_sb[:, cb:cb + 1])
                vpan.append(vt)

            for tt in range(4):
                ps_o = psum.tile([P, D], F32, tag="ps", bufs=6)
                nc.tensor.matmul(ps_o[:], ones_bf[:], b2row_bf[:],
                                 start=True, stop=False)
                for cb in range(CB):
                    nc.tensor.matmul(
                        ps_o[:], vpan[cb][:, tt * P:(tt + 1) * P], w2t[cb][:],
                        start=False, stop=(cb == CB - 1))
                o_sb = outp.tile([P, D], F32, tag="o_sb")
                nc.scalar.activation(o_sb[:], ps_o[:], ACTF.Copy)
                t0 = tp * 512 + tt * P
                nc.sync.dma_start(out_d[b, t0:t0 + P, :], o_sb[:])


_NC_CACHE = None


def _get_program():
    global _NC_CACHE
    if _NC_CACHE is None:
        nc = bacc.Bacc("TRN2", target_bir_lowering=False, debug=False)
        with tile.TileContext(nc) as tc, ExitStack() as ctx:
            _build_kernel(ctx, tc)
        nc.compile()
        _NC_CACHE = nc
    return _NC_CACHE


def _bf16_bits(a):
    """RNE float32 -> bf16 bit pattern, as uint16."""
    u = np.ascontiguousarray(a, np.float32).view(np.uint32)
    rounded = u + 0x7FFF + ((u >> 16) & 1)
    return (rounded >> 16).astype(np.uint16)


def host_prep(ln_g, ln_b, w1, b1, dw, dwb, alpha, w2, b2):
    f32 = np.float32
    w1 = np.asarray(w1, f32)
    w2 = np.asarray(w2, f32)
    b1 = np.asarray(b1, f32)
    ln_g = np.asarray(ln_g, f32)
    ln_b = np.asarray(ln_b, f32)
    dw = np.asarray(dw, f32)
    dwb = np.asarray(dwb, f32)
    alpha = np.asarray(alpha, f32)
    b2 = np.asarray(b2, f32)
    w1t = np.ascontiguousarray((w1 * ln_g[None, :]).T)            # [D, 2I]
    b1e = b1 + w1 @ ln_b                                          # [2I]
    b1p = np.ascontiguousarray(b1e.reshape(ETILES, P).T)          # [128, 16]
    w2t = np.ascontiguousarray(w2.T)                              # [I, D]
    dwf = dw[:, 0, :].reshape(CB, P, K)
    dwsb = np.ascontiguousarray(dwf.transpose(1, 0, 2).reshape(P, CB * K))
    # diag[p, cb, k, q] = (p==q) * dw[cb*128+p, k], bf16 bits
    dia = np.einsum('pq,cpk->pckq', np.eye(P, dtype=f32), dwf)
    diagw = np.ascontiguousarray(_bf16_bits(dia).reshape(P, CB * K * P))
    return {
        "w1t": w1t, "w2t": w2t, "b1p": b1p, "dwsb": dwsb,
        "dwbp": np.ascontiguousarray(dwb.reshape(CB, P).T),
        "alphap": np.ascontiguousarray(alpha.reshape(CB, P).T),
        "b2row": np.ascontiguousarray(b2[None, :]),
        "diagw": diagw,
    }


def kernel(x, ln_g, ln_b, w1, b1, dw, dwb, alpha, w2, b2, _trace=False):
    nc = _get_program()
    x = np.ascontiguousarray(x, np.float32)
    shared = host_prep(ln_g, ln_b, w1, b1, dw, dwb, alpha, w2, b2)
    in_maps = [
        {"x": x[c * BPC:(c + 1) * BPC], **shared} for c in range(NCORES)
    ]
    res = run_bass_kernel_spmd(nc, in_maps, core_ids=list(range(NCORES)),
                               trace=_trace)
    out = np.concatenate([res.results[c]["out"] for c in range(NCORES)], axis=0)
    if _trace:
        kernel.last_results = res
    return out


# revision 10
# speedup vs baseline: 1.8412x; 1.0179x over previous
"""Trainium2 Bass kernel for nn_ConvModule: LN -> 1x1 conv (D->2I) -> SwiGLU
-> depthwise conv (K=31) -> PReLU -> 1x1 conv (I->D).

Sharding: data-parallel over batch, 2 batches per core across 8 cores.

v3:
  - Host numpy preprocessing: w1/w2 transposed, ln_g folded into W1, ln_b
    into b1, and the 8 per-channel-block diagonal tap matrices prebuilt as
    bf16 (DMA'd once; no per-batch diag building on any engine).
  - GEMM path all bf16 (fp32 moving operands stream at half PE rate).
  - SwiGLU epilogue: ACT Silu + ACT Identity-with-bias + all-bf16 DVE mul.
  - Conv: PE diag matmuls for 18 taps; 13 odd taps on DVE as an all-bf16
    mult/accumulate chain at 4B-aligned strip offsets; PReLU+dwb in one ACT
    Prelu op. tp-outer loop so GEMM2 interleaves with the conv.
  - DVE ops keep uniform dtypes per op (mixed f32/bf16 operand combos fall
    into a slow DVE path; measured 1.8us vs 0.33us on [128,512]).
"""

import sys

sys.path.insert(0, "/opt/trn_rl_repo")

from contextlib import ExitStack

import numpy as np

import concourse.bacc as bacc
import concourse.tile as tile
from concourse import mybir
from concourse.masks import make_identity
from concourse.bass_utils import run_bass_kernel_spmd

B, T, D, I, K = 16, 2048, 512, 1024, 31
NCORES = 8
BPC = B // NCORES  # batches per core
E = 2 * I  # 2048
TP = T // 512  # time panels per batch (4)
ETILES = E // 128  # 16
CB = I // 128  # channel blocks (8)
DCH = D // 128  # d chunks (4)
PADL = 16  # tap k reads strip offset k+1, so odd k -> 4B-aligned bf16 slice
PADR = 16
STRIPW = PADL + T + PADR  # 2080
NDVE = 9
DVE_TAPS = [2 * i + 1 for i in range(NDVE)]  # 1,3,...,17
PE_TAPS = [k for k in range(K) if k not in DVE_TAPS]  # 22 taps

F32 = mybir.dt.float32
BF16 = mybir.dt.bfloat16
U16 = mybir.dt.uint16
ALU = mybir.AluOpType
ACTF = mybir.ActivationFunctionType
P = 128


def _build_kernel(ctx, tc):
    nc = tc.nc
    x_d = nc.dram_tensor("x", [BPC, T, D], F32, kind="ExternalInput").ap()
    w1t_d = nc.dram_tensor("w1t", [D, E], F32, kind="ExternalInput").ap()
    w2t_d = nc.dram_tensor("w2t", [I, D], F32, kind="ExternalInput").ap()
    b1p_d = nc.dram_tensor("b1p", [P, ETILES], F32, kind="ExternalInput").ap()
    dwsb_d = nc.dram_tensor("dwsb", [P, CB * K], F32, kind="ExternalInput").ap()
    dwb_d = nc.dram_tensor("dwbp", [P, CB], F32, kind="ExternalInput").ap()
    alpha_d = nc.dram_tensor("alphap", [P, CB], F32, kind="ExternalInput").ap()
    b2_d = nc.dram_tensor("b2row", [1, D], F32, kind="ExternalInput").ap()
    # prebuilt bf16 diagonal tap matrices, as uint16 bit patterns
    diag_d = nc.dram_tensor("diagw", [P, CB * K * P], U16,
                            kind="ExternalInput").ap()
    out_d = nc.dram_tensor("out", [BPC, T, D], F32, kind="ExternalOutput").ap()

    const = ctx.enter_context(tc.tile_pool(name="const", bufs=1))
    psum = ctx.enter_context(tc.tile_pool(name="psum", bufs=8, space="PSUM"))

    ident_bf = const.tile([P, P], BF16, tag="ident_bf")
    make_identity(nc, ident_bf[:])

    # ---- parameter tiles ----
    b1p = const.tile([P, ETILES], F32, tag="b1p")
    nc.sync.dma_start(b1p[:], b1p_d)
    alpha_sb = const.tile([P, CB], F32, tag="alpha_sb")
    nc.sync.dma_start(alpha_sb[:], alpha_d)
    dwb_sb = const.tile([P, CB], F32, tag="dwb_sb")
    nc.sync.dma_start(dwb_sb[:], dwb_d)
    dw_sb = const.tile([P, CB * K], F32, tag="dw_sb")
    nc.sync.dma_start(dw_sb[:], dwsb_d)
    eps_t = const.tile([P, 1], F32, tag="eps_t")
    nc.vector.memset(eps_t[:], 1e-5)
    ones_bf = const.tile([1, P], BF16, tag="ones_bf")
    nc.vector.memset(ones_bf[:], 1.0)

    diag = [const.tile([P, K * P], BF16, tag=f"diag{cb}", name=f"diag{cb}")
            for cb in range(CB)]

    def load_diags():
        for cb in range(CB):
            nc.sync.dma_start(
                diag[cb][:].bitcast(U16),
                diag_d[:, cb * K * P:(cb + 1) * K * P])

    w1t = [const.tile([P, E], BF16, tag=f"w1t{j}", name=f"w1t{j}") for j in range(DCH)]
    w2t = [const.tile([P, D], BF16, tag=f"w2t{i}", name=f"w2t{i}") for i in range(CB)]
    b2row_bf = const.tile([1, D], BF16, tag="b2row_bf")
    with tc.tile_pool(name="setup", bufs=2) as setup:
        b2f = setup.tile([1, D], F32, tag="b2f", bufs=1)
        nc.sync.dma_start(b2f[:], b2_d)
        nc.vector.tensor_copy(b2row_bf[:], b2f[:])
        for j in range(DCH):
            wst = setup.tile([P, E], F32, tag="wst", bufs=2)
            nc.sync.dma_start(wst[:], w1t_d[j * P:(j + 1) * P, :])
            nc.vector.tensor_copy(w1t[j][:], wst[:])
        for i in range(CB):
            wst2 = setup.tile([P, D], F32, tag="wst2", bufs=2)
            nc.sync.dma_start(wst2[:], w2t_d[i * P:(i + 1) * P, :])
            nc.vector.tensor_copy(w2t[i][:], wst2[:])

    # ---- pools for the main loop ----
    xpool = ctx.enter_context(tc.tile_pool(name="xpool", bufs=2))
    xnpool = ctx.enter_context(tc.tile_pool(name="xnpool", bufs=4))
    xbfpool = ctx.enter_context(tc.tile_pool(name="xbfpool", bufs=5))
    stat = ctx.enter_context(tc.tile_pool(name="stat", bufs=20))
    scr = ctx.enter_context(tc.tile_pool(name="scr", bufs=2))
    xnt = ctx.enter_context(tc.tile_pool(name="xnt", bufs=12))
    sw = ctx.enter_context(tc.tile_pool(name="sw", bufs=4))
    strips = ctx.enter_context(tc.tile_pool(name="strips", bufs=8))
    vact = ctx.enter_context(tc.tile_pool(name="vact", bufs=12))
    wsbp = ctx.enter_context(tc.tile_pool(name="wsbp", bufs=4))
    outp = ctx.enter_context(tc.tile_pool(name="outp", bufs=3))

    def load_x_panel(b, tp):
        tiles = []
        for tt in range(4):
            t0 = tp * 512 + tt * P
            x_t = xpool.tile([P, D], F32, tag="x", bufs=8, name=f"x_{b}_{tp}_{tt}")
            nc.scalar.dma_start(x_t[:], x_d[b, t0:t0 + P, :])
            tiles.append(x_t)
        return tiles

    def emit_ln_panel(b, tp, x_tiles):
        """LayerNorm + PE-transpose for one 512-token panel -> xnT bf16."""
        means, negvs, stdvs = [], [], []
        for tt in range(4):
            x_t = x_tiles[tt]
            mean = stat.tile([P, 1], F32, tag="mean")
            ex2 = stat.tile([P, 1], F32, tag="ex2")
            xcp = scr.tile([P, D], BF16, tag="xscr")
            nc.scalar.activation(xcp[:], x_t[:], ACTF.Identity,
                                 scale=1.0 / D, accum_out=mean[:])
            xsq = scr.tile([P, D], BF16, tag="xscr")
            nc.scalar.activation(xsq[:], x_t[:], ACTF.Square,
                                 scale=1.0 / np.sqrt(D), accum_out=ex2[:])
            negv = stat.tile([P, 1], F32, tag="negv")
            nc.vector.scalar_tensor_tensor(
                negv[:], mean[:], mean[:], ex2[:],
                op0=ALU.mult, op1=ALU.subtract)
            means.append(mean)
            negvs.append(negv)
        for tt in range(4):
            stdv = stat.tile([P, 1], F32, tag="stdv")
            nc.scalar.activation(stdv[:], negvs[tt][:], ACTF.Sqrt,
                                 scale=-1.0, bias=eps_t[:])
            stdvs.append(stdv)
        xn_tiles = []
        for tt in range(4):
            rstd = stat.tile([P, 1], F32, tag="rstd")
            nc.vector.reciprocal(rstd[:], stdvs[tt][:])
            xn_t = xnpool.tile([P, D], F32, tag="xn")
            nc.vector.tensor_scalar(
                xn_t[:], x_tiles[tt][:], means[tt][:], rstd[:],
                op0=ALU.subtract, op1=ALU.mult)
            xn_bf = xbfpool.tile([P, D], BF16, tag="xnbf")
            nc.vector.tensor_copy(xn_bf[:], xn_t[:])
            xn_tiles.append(xn_bf)
        xnt_p = []
        for j in range(DCH):
            ptr = psum.tile([P, 512], BF16, tag="ps_tr", bufs=2)
            for tt in range(4):
                nc.tensor.transpose(
                    ptr[:, tt * P:(tt + 1) * P],
                    xn_tiles[tt][:, j * P:(j + 1) * P], ident_bf[:])
            xt = xnt.tile([P, 512], BF16, tag="xnt")
            nc.vector.tensor_copy(xt[:], ptr[:])
            xnt_p.append(xt)
        return xnt_p

    xq = {(0, 0): load_x_panel(0, 0)}
    xnt_cache = {(0, 0): emit_ln_panel(0, 0, xq.pop((0, 0)))}
    xq[(0, 1)] = load_x_panel(0, 1)
    xnt_cache[(0, 1)] = emit_ln_panel(0, 1, xq.pop((0, 1)))

    for b in range(BPC):
        # ---------- LN + GEMM1 + SwiGLU ----------
        strip = []
        for cb in range(CB):
            s = strips.tile([P, STRIPW], BF16, tag="strip")
            nc.gpsimd.memset(s[:, 0:PADL], 0.0)
            nc.gpsimd.memset(s[:, PADL + T:STRIPW], 0.0)
            strip.append(s)

        for tp in range(TP):
            if tp + 1 < TP:
                if (b, tp + 1) not in xnt_cache:
                    xq[(b, tp + 1)] = load_x_panel(b, tp + 1)
            elif b + 1 < BPC:
                xq[(b + 1, 0)] = load_x_panel(b + 1, 0)
            if (b, tp) in xnt_cache:
                xnt_p = xnt_cache.pop((b, tp))
            else:
                xnt_p = emit_ln_panel(b, tp, xq.pop((b, tp)))

            for i in range(CB):
                ps_a = psum.tile([P, 512], F32, tag="ps", bufs=6)
                ps_g = psum.tile([P, 512], F32, tag="ps", bufs=6)
                for j in range(DCH):
                    nc.tensor.matmul(
                        ps_a[:], w1t[j][:, i * P:(i + 1) * P], xnt_p[j][:],
                        start=(j == 0), stop=(j == DCH - 1))
                for j in range(DCH):
                    ii = i + CB
                    nc.tensor.matmul(
                        ps_g[:], w1t[j][:, ii * P:(ii + 1) * P], xnt_p[j][:],
                        start=(j == 0), stop=(j == DCH - 1))
                # u = (a + b1a) * silu(g + b1g)
                s_sb = sw.tile([P, 512], BF16, tag="s_sb")
                nc.scalar.activation(
                    s_sb[:], ps_g[:], ACTF.Silu, bias=b1p[:, i + CB:i + CB + 1])
                nc.vector.scalar_tensor_tensor(
                    strip[i][:, PADL + tp * 512:PADL + (tp + 1) * 512],
                    ps_a[:], b1p[:, i:i + 1], s_sb[:],
                    op0=ALU.add, op1=ALU.mult)

        if b == 0:
            load_diags()

        # ---------- depthwise conv + PReLU + GEMM2, per time panel ----------
        for tp in range(TP):
            vpan = []
            for cb in range(CB):
                ps_c = psum.tile([P, 512], F32, tag="ps", bufs=6)
                for ti, tap in enumerate(PE_TAPS):
                    off = tp * 512 + tap + 1
                    nc.tensor.matmul(
                        ps_c[:], diag[cb][:, tap * P:(tap + 1) * P],
                        strip[cb][:, off:off + 512],
                        start=(ti == 0), stop=(ti == len(PE_TAPS) - 1))
                # DVE taps: seed product on ACT, f32 mult-add chain on DVE
                td = wsbp.tile([P, 512], F32, tag="td")
                k0 = DVE_TAPS[0]
                nc.scalar.activation(
                    td[:], strip[cb][:, tp * 512 + k0 + 1:tp * 512 + k0 + 513],
                    ACTF.Identity, scale=dw_sb[:, cb * K + k0:cb * K + k0 + 1])
                for tap in DVE_TAPS[1:]:
                    off = tp * 512 + tap + 1
                    nc.vector.scalar_tensor_tensor(
                        td[:], strip[cb][:, off:off + 512],
                        dw_sb[:, cb * K + tap:cb * K + tap + 1], td[:],
                        op0=ALU.mult, op1=ALU.add)
                w_sb = wsbp.tile([P, 512], F32, tag="w_sb")
                nc.vector.tensor_add(w_sb[:], ps_c[:], td[:])
                # v = prelu(w + dwb) with per-channel alpha, in one ACT op
                vt = vact.tile([P, 512], BF16, tag="vact")
                nc.scalar.activation(
                    vt[:], w_sb[:], ACTF.Prelu,
                    bias=dwb_sb[:, cb:cb + 1], alpha=alpha_sb[:, cb:cb + 1])
                vpan.append(vt)

            for tt in range(4):
                ps_o = psum.tile([P, D], F32, tag="ps", bufs=6)
                nc.tensor.matmul(ps_o[:], ones_bf[:], b2row_bf[:],
                                 start=True, stop=False)
                for cb in range(CB):
                    nc.tensor.matmul(
                        ps_o[:], vpan[cb][:, tt * P:(tt + 1) * P], w2t[cb][:],
                        start=False, stop=(cb == CB - 1))
                o_sb = outp.tile([P, D], F32, tag="o_sb")
                nc.scalar.activation(o_sb[:], ps_o[:], ACTF.Copy)
                t0 = tp * 512 + tt * P
                nc.sync.dma_start(out_d[b, t0:t0 + P, :], o_sb[:])


_NC_CACHE = None


def _get_program():
    global _NC_CACHE
    if _NC_CACHE is None:
        nc = bacc.Bacc("TRN2", target_bir_lowering=False, debug=False)
        with tile.TileContext(nc) as tc, ExitStack() as ctx:
            _build_kernel(ctx, tc)
        nc.compile()
        _NC_CACHE = nc
    return _NC_CACHE


def _bf16_bits(a):
    """RNE float32 -> bf16 bit pattern, as uint16."""
    u = np.ascontiguousarray(a, np.float32).view(np.uint32)
    rounded = u + 0x7FFF + ((u >> 16) & 1)
    return (rounded >> 16).astype(np.uint16)


def host_prep(ln_g, ln_b, w1, b1, dw, dwb, alpha, w2, b2):
    f32 = np.float32
    w1 = np.asarray(w1, f32)
    w2 = np.asarray(w2, f32)
    b1 = np.asarray(b1, f32)
    ln_g = np.asarray(ln_g, f32)
    ln_b = np.asarray(ln_b, f32)
    dw = np.asarray(dw, f32)
    dwb = np.asarray(dwb, f32)
    alpha = np.asarray(alpha, f32)
    b2 = np.asarray(b2, f32)
    w1t = np.ascontiguousarray((w1 * ln_g[None, :]).T)            # [D, 2I]
    b1e = b1 + w1 @ ln_b                                          # [2I]
    b1p = np.ascontiguousarray(b1e.reshape(ETILES, P).T)          # [128, 16]
    w2t = np.ascontiguousarray(w2.T)                              # [I, D]
    dwf = dw[:, 0, :].reshape(CB, P, K)
    dwsb = np.ascontiguousarray(dwf.transpose(1, 0, 2).reshape(P, CB * K))
    # diag[p, cb, k, q] = (p==q) * dw[cb*128+p, k], bf16 bits
    dia = np.einsum('pq,cpk->pckq', np.eye(P, dtype=f32), dwf)
    diagw = np.ascontiguousarray(_bf16_bits(dia).reshape(P, CB * K * P))
    return {
        "w1t": w1t, "w2t": w2t, "b1p": b1p, "dwsb": dwsb,
        "dwbp": np.ascontiguousarray(dwb.reshape(CB, P).T),
        "alphap": np.ascontiguousarray(alpha.reshape(CB, P).T),
        "b2row": np.ascontiguousarray(b2[None, :]),
        "diagw": diagw,
    }


def kernel(x, ln_g, ln_b, w1, b1, dw, dwb, alpha, w2, b2, _trace=False):
    nc = _get_program()
    x = np.ascontiguousarray(x, np.float32)
    shared = host_prep(ln_g, ln_b, w1, b1, dw, dwb, alpha, w2, b2)
    in_maps = [
        {"x": x[c * BPC:(c + 1) * BPC], **shared} for c in range(NCORES)
    ]
    res = run_bass_kernel_spmd(nc, in_maps, core_ids=list(range(NCORES)),
                               trace=_trace)
    out = np.concatenate([res.results[c]["out"] for c in range(NCORES)], axis=0)
    if _trace:
        kernel.last_results = res
    return out
